# revision 1
# baseline (speedup 1.0000x reference)
"""Trainium2 Bass kernel for vq_codebook argmin (nn_GUMSampler).

Per pixel p (4M pixels), compute d2[v] = ||z_p - vertex_v||^2 for 16 vertices
in R^15, output argmin index (int32) and min distance (f32).

Strategy (per NeuronCore, pixels sharded 8 ways):
  - Pixels are packed 8-per-PSUM-column ("groups"): PSUM row 8v+g holds
    d2 of vertex v for pixel-group g.  Full d2 is accumulated in PSUM by
    three chained matmuls sharing one 128-row output:
       mm_z :  lhsT[15g+c, 8v+g] = -2*V[v,c]     rhs = z tile     (K=120)
       mm_1 :  lhsT[g,     8v+g] = |V_v|^2       rhs = ones       (K=8)
       mm_sq:  lhsT[15g+c, 8v+g] = 1.0           rhs = z^2 tile   (K=120)
  - Vertex index is bit-packed into the 4 low mantissa bits of d2
    (d2 >= ~4 always, so a 16-ulp perturbation is ~1e-6 relative):
       packed = (bits(d2) & ~15) | v
    f32 min over packed values then yields min-d2 AND its argmin, with
    jnp.argmin's first-index tie-break (smaller v == smaller packed).
  - 16->1 min over the v rows is a partition-halving tree.  The Neuron
    compiler requires equal base partitions when both tensor_tensor inputs
    are SBUF (and 32-aligned bases for any compute read), so each level is
    (shift-copy upper half to partition 0) + (aligned tensor_tensor min).
    Shift copies are spread across ACT (32-aligned ones) and SBUF->SBUF DMA
    (sub-32 bases, which compute engines cannot read); mins run on DVE.
    ACT also pre-copies PSUM->SBUF so the pack runs in DVE 2x mode.
  - Epilogue per 4 tiles: idx = packed & 15, dmin = sqrt(packed) (the 4
    index bits perturb d2 by <= 15 ulp, ~2e-6 relative, below tolerance).
"""

import sys

sys.path.insert(0, "/opt/trn_rl_repo")

from contextlib import ExitStack

import numpy as np

import concourse.bacc as bacc
import concourse.tile as tile
from concourse import mybir
from concourse.bass_utils import run_bass_kernel_spmd

F32 = mybir.dt.float32
I32 = mybir.dt.int32

K = 16          # vertices
C = 15          # channels (K-1)
G = 8           # pixel groups per PSUM column
EFF = 1024      # pixels per iteration per group (2 PSUM banks)
N_CORES = 8
LX = LY = 2048
N_TOTAL = LX * LY
N_LOC = N_TOTAL // N_CORES          # 524288 pixels per core
N_ITERS = N_LOC // (G * EFF)        # 64
GBLK = N_LOC // G                   # 65536 pixels per group block

_CACHE = {}


def build_nc(n_iters=N_ITERS):
    n_loc = n_iters * G * EFF
    gblk = n_loc // G
    nc = bacc.Bacc("TRN2", target_bir_lowering=False, debug=False)

    z_d = nc.dram_tensor("z", [C, n_loc], F32, kind="ExternalInput")
    w1z_d = nc.dram_tensor("w1z", [C * G, 128], F32, kind="ExternalInput")
    w1o_d = nc.dram_tensor("w1o", [G, 128], F32, kind="ExternalInput")
    wsq_d = nc.dram_tensor("wsq", [C * G, 128], F32, kind="ExternalInput")
    vvec_d = nc.dram_tensor("vvec", [128, 1], I32, kind="ExternalInput")
    idx_d = nc.dram_tensor("idx", [n_loc], I32, kind="ExternalOutput")
    dmin_d = nc.dram_tensor("dmin", [n_loc], F32, kind="ExternalOutput")

    AND_MASK = -16  # 0xFFFFFFF0
    MIN = mybir.AluOpType.min

    with tile.TileContext(nc) as tc, ExitStack() as ctx:
        cpool = ctx.enter_context(tc.tile_pool(name="consts", bufs=1))
        w1z_s = cpool.tile([C * G, 128], F32)
        w1o_s = cpool.tile([G, 128], F32)
        wsq_s = cpool.tile([C * G, 128], F32)
        vvec_s = cpool.tile([128, 1], I32)
        ones_s = cpool.tile([G, EFF], F32)
        nc.sync.dma_start(w1z_s[:], w1z_d[:])
        nc.sync.dma_start(w1o_s[:], w1o_d[:])
        nc.sync.dma_start(wsq_s[:], wsq_d[:])
        nc.sync.dma_start(vvec_s[:], vvec_d[:])
        nc.vector.memset(ones_s[:], 1.0)

        zpool = ctx.enter_context(tc.tile_pool(name="z", bufs=4))
        sqpool = ctx.enter_context(tc.tile_pool(name="zsq", bufs=2))
        pspool = ctx.enter_context(tc.tile_pool(name="psum1", bufs=2, space="PSUM"))
        psbpool = ctx.enter_context(tc.tile_pool(name="psb", bufs=3))
        pkpool = ctx.enter_context(tc.tile_pool(name="pk", bufs=3))
        c1pool = ctx.enter_context(tc.tile_pool(name="c1", bufs=3))
        t1pool = ctx.enter_context(tc.tile_pool(name="t1", bufs=3))
        c2pool = ctx.enter_context(tc.tile_pool(name="c2", bufs=3))
        t2pool = ctx.enter_context(tc.tile_pool(name="t2", bufs=3))
        c3pool = ctx.enter_context(tc.tile_pool(name="c3", bufs=3))
        t3pool = ctx.enter_context(tc.tile_pool(name="t3", bufs=3))
        c4pool = ctx.enter_context(tc.tile_pool(name="c4", bufs=3))
        bscpool = ctx.enter_context(tc.tile_pool(name="bsc", bufs=2))
        dmpool = ctx.enter_context(tc.tile_pool(name="dm", bufs=2))
        idxpool = ctx.enter_context(tc.tile_pool(name="idx", bufs=2))

        idx_view = idx_d[:].rearrange(
            "(g bb j f) -> bb j g f", g=G, bb=n_iters // 2, j=4, f=512
        )
        dmin_view = dmin_d[:].rearrange(
            "(g bb j f) -> bb j g f", g=G, bb=n_iters // 2, j=4, f=512
        )

        bsc = None
        for i in range(n_iters):
            # ---- load z tile: partitions 15g+c, free = EFF pixels ----
            z_t = zpool.tile([C * G, EFF], F32)
            for g in range(G):
                off = g * gblk + i * EFF
                nc.sync.dma_start(
                    z_t[C * g : C * g + C, :], z_d[:, off : off + EFF]
                )

            # ---- z^2 on ACT ----
            zsq = sqpool.tile([C * G, EFF], F32)
            nc.scalar.square(zsq[:], z_t[:])

            # ---- full d2 into PSUM via 3 accumulating matmuls ----
            ps = pspool.tile([128, EFF], F32)
            for h in (0, 1):
                sl = slice(512 * h, 512 * h + 512)
                nc.tensor.matmul(ps[:, sl], w1z_s[:], z_t[:, sl], start=True, stop=False)
                nc.tensor.matmul(ps[:, sl], w1o_s[:], ones_s[:, sl], start=False, stop=False)
                nc.tensor.matmul(ps[:, sl], wsq_s[:], zsq[:, sl], start=False, stop=True)

            # ---- ACT copies PSUM->SBUF so the pack runs in DVE 2x mode ----
            psb = psbpool.tile([128, EFF], F32)
            nc.scalar.copy(psb[:], ps[:])

            # ---- pack: (bits(d2) & ~15) | v   [v = row >> 3] ----
            pk = pkpool.tile([128, EFF], F32)
            nc.vector.tensor_scalar(
                pk[:].bitcast(I32), psb[:].bitcast(I32), AND_MASK, vvec_s[:],
                op0=mybir.AluOpType.bitwise_and, op1=mybir.AluOpType.bitwise_or,
            )

            # ---- min tree: 128 rows (8v+g) -> 8 rows (g) ----
            c1 = c1pool.tile([64, EFF], F32)
            nc.sync.dma_start(c1[:], pk[64:128, :])
            t1 = t1pool.tile([64, EFF], F32)
            nc.vector.tensor_tensor(t1[:], pk[0:64, :], c1[:], MIN)

            c2 = c2pool.tile([32, EFF], F32)
            nc.scalar.copy(c2[:], t1[32:64, :])
            t2 = t2pool.tile([32, EFF], F32)
            nc.vector.tensor_tensor(t2[:], t1[0:32, :], c2[:], MIN)

            # partitions 16:32 / 8:16 are not 32-aligned -> compute engines
            # cannot read them; move with SBUF->SBUF DMA instead
            c3 = c3pool.tile([16, EFF], F32)
            nc.sync.dma_start(c3[:], t2[16:32, :])
            t3 = t3pool.tile([16, EFF], F32)
            nc.vector.tensor_tensor(t3[:], t2[0:16, :], c3[:], MIN)

            c4 = c4pool.tile([8, EFF], F32)
            nc.sync.dma_start(c4[:], t3[8:16, :])

            if i % 2 == 0:
                bsc = bscpool.tile([128, 512], F32)
            for h in (0, 1):
                sl = slice(512 * h, 512 * h + 512)
                j = 2 * (i % 2) + h
                nc.vector.tensor_tensor(
                    bsc[32 * j : 32 * j + 8, :], t3[0:8, sl], c4[:, sl], MIN
                )

            # ---- epilogue every 2 iterations (4 tiles of 512) ----
            if i % 2 == 1:
                b = i // 2
                # sqrt directly on packed values: the 4 index bits perturb
                # d2 by <= 15 ulp (~2e-6 relative) which is below tolerance
                dm = dmpool.tile([128, 512], F32)
                nc.scalar.sqrt(dm[:], bsc[:])
                ix = idxpool.tile([128, 512], I32)
                nc.vector.tensor_scalar(
                    ix[:], bsc[:].bitcast(I32), 15, None,
                    op0=mybir.AluOpType.bitwise_and,
                )
                dm_dst4 = dmin_view[b : b + 1].rearrange("one j g f -> (one j) g f")
                ix_dst4 = idx_view[b : b + 1].rearrange("one j g f -> (one j) g f")
                for j in range(4):
                    dm_dst = dm_dst4[j : j + 1].rearrange("one g f -> (one g) f")
                    ix_dst = ix_dst4[j : j + 1].rearrange("one g f -> (one g) f")
                    nc.sync.dma_start(dm_dst, dm[32 * j : 32 * j + 8, :])
                    nc.sync.dma_start(ix_dst, ix[32 * j : 32 * j + 8, :])

    nc.compile()
    return nc


def _weights(vertices):
    V = np.asarray(vertices, dtype=np.float32)          # (16, 15)
    vv = (V.astype(np.float64) ** 2).sum(1).astype(np.float32)
    w1z = np.zeros((C * G, 128), dtype=np.float32)
    w1o = np.zeros((G, 128), dtype=np.float32)
    wsq = np.zeros((C * G, 128), dtype=np.float32)
    for g in range(G):
        # output column 8v+g
        w1z[C * g : C * g + C, g::G] = -2.0 * V.T        # (15, 16)
        w1o[g, g::G] = vv
        wsq[C * g : C * g + C, g::G] = 1.0
    vvec = (np.arange(128, dtype=np.int32) >> 3).reshape(128, 1)
    return w1z, w1o, wsq, vvec.astype(np.int32)


def kernel(z, vertices):
    z = np.ascontiguousarray(np.asarray(z, dtype=np.float32))
    k, lx, ly = K, z.shape[1], z.shape[2]
    n = lx * ly
    z_fl = z.reshape(C, n)
    n_loc = n // N_CORES

    if "nc" not in _CACHE:
        _CACHE["nc"] = build_nc()
    nc = _CACHE["nc"]

    w1z, w1o, wsq, vvec = _weights(vertices)
    in_maps = []
    for c in range(N_CORES):
        in_maps.append(
            {
                "z": np.ascontiguousarray(z_fl[:, c * n_loc : (c + 1) * n_loc]),
                "w1z": w1z,
                "w1o": w1o,
                "wsq": wsq,
                "vvec": vvec,
            }
        )
    res = run_bass_kernel_spmd(nc, in_maps, list(range(N_CORES)))
    X = np.concatenate([res.results[c]["idx"] for c in range(N_CORES)])
    dmin = np.concatenate([res.results[c]["dmin"] for c in range(N_CORES)])
    return X.reshape(lx, ly), dmin.reshape(lx, ly)


if __name__ == "__main__":
    rng = np.random.default_rng(0)
    z = rng.standard_normal((C, 64, 64), dtype=np.float32)
    print("smoke build only")



# revision 20
# speedup vs baseline: 1.7774x; 1.7774x over previous
"""Trainium2 Bass kernel for vq_codebook argmin (nn_GUMSampler).

Per pixel p, d2[v] = ||z_p - vertex_v||^2 over 16 vertices in R^15;
outputs argmin index (int32) and min distance (f32).

Layout (per core; pixels sharded 8 ways across cores, then 32 pixel
groups g3 per core with gblk = n_loc/32 pixels each; two 512-px macros
are processed together as one 1024-wide "pair"):
  - Four PSUM tiles P_i [128, 512] per macro; tile i holds vertices
    {4i+v', v'=0..3} at row 32v'+g3.  Each accumulates 4 fp32r matmuls
    (one per group-octet, K=121: ones-row 0 carries ||v||^2 on octet 0,
    then rows 1+15g+c of the shared z tile).  PSUM holds
    q = ||v||^2 - 2 v.z   (||z||^2 is argmin-irrelevant, added at the
    end from a separate small PSUM tile).
  - zz = ||z||^2 via 4 accumulating bf16 matmuls of z^2 (ACT square,
    bf16) into ZZ [32, 512] per macro.
  - ACT stages each P_i into the halves of an SBUF pair tile [128,1024];
    pack runs on DVE in 2x mode at pair width:
      packed = (bits(q) & ~15) | vertex_id   (id = 4v'+i per partition)
    f32 min then carries value AND argmin; ties pick the smaller id.
  - Min tree at pair width, every level 32-aligned:
      t1a=min(pk0,pk1) t1b=min(pk2,pk3) t2=min(t1a,t1b)  [128,1024]
      t3=min(t2[0:64], dma-shift t2[64:128])              [64,1024]
      t4=min(t3[0:32], dma-shift t3[32:64])               [32,1024]
    (walrus + HW require equal SBUF base partitions for tensor_tensor
    inputs, so the sub-64 shifts are cross-partition copies on the
    otherwise-idle GPSIMD engine.)
  - Epilogue: idx = packed & 15 (DVE 2x, pair width); dmin^2 = packed +
    zz per macro half (id junk bits <= 15 ulp, ~2e-6 rel), sqrt on ACT.
  - One z DMA per pair (one strided DMA, 480 x 4KB descriptors), one
    idx/dmin DMA per 2 pairs.
"""

import sys

sys.path.insert(0, "/opt/trn_rl_repo")

from contextlib import ExitStack

import numpy as np

import concourse.bacc as bacc
import concourse.tile as tile
from concourse import mybir
from concourse.bass_utils import run_bass_kernel_spmd

F32 = mybir.dt.float32
F32R = mybir.dt.float32r
BF16 = mybir.dt.bfloat16
I32 = mybir.dt.int32

K = 16
C = 15
G3 = 32          # pixel groups per core
EFF = 512        # pixels per macro per group
N_CORES = 8
LX = LY = 2048
N_TOTAL = LX * LY
N_LOC = N_TOTAL // N_CORES       # 524288
GBLK = N_LOC // G3               # 16384
N_MACROS = GBLK // EFF           # 32

AND_MASK = -16
MIN = mybir.AluOpType.min
ADD = mybir.AluOpType.add

_CACHE = {}


def build_nc(n_macros=N_MACROS):
    assert n_macros % 4 == 0
    gblk = n_macros * EFF
    n_loc = G3 * gblk
    n_pairs = n_macros // 2
    PW = 2 * EFF                 # pair width (1024)
    nc = bacc.Bacc("TRN2", target_bir_lowering=False, debug=False)

    z_d = nc.dram_tensor("z", [C, n_loc], F32R, kind="ExternalInput")
    w_d = nc.dram_tensor("w", [C * 8 + 1, 16 * 128], F32R, kind="ExternalInput")
    wz_d = nc.dram_tensor("wz", [C * 8 + 1, 128], BF16, kind="ExternalInput")
    vvec_d = nc.dram_tensor("vvec", [128, 4], I32, kind="ExternalInput")
    idx_d = nc.dram_tensor("idx", [n_loc], I32, kind="ExternalOutput")
    dmin_d = nc.dram_tensor("dmin", [n_loc], F32, kind="ExternalOutput")

    with tile.TileContext(nc) as tc, ExitStack() as ctx:
        cpool = ctx.enter_context(tc.tile_pool(name="consts", bufs=1))
        w_s = cpool.tile([C * 8 + 1, 16 * 128], F32R)
        wz_s = cpool.tile([C * 8 + 1, 128], BF16)
        vvec_s = cpool.tile([128, 4], I32)
        nc.sync.dma_start(w_s[:], w_d[:])
        nc.sync.dma_start(wz_s[:], wz_d[:])
        nc.sync.dma_start(vvec_s[:], vvec_d[:])

        # persistent double-buffered z pair tiles; row 0 = 1.0 (set once)
        zbufs = [
            cpool.tile([C * 8 + 1, 4 * PW], F32R, name=f"zb{k}") for k in range(2)
        ]
        for zb in zbufs:
            nc.gpsimd.memset(zb[0:1, :].bitcast(F32), 1.0)

        zsqpool = ctx.enter_context(tc.tile_pool(name="zsq", bufs=2))
        ppool = ctx.enter_context(tc.tile_pool(name="psum", bufs=6, space="PSUM"))
        zzpool = ctx.enter_context(tc.tile_pool(name="zz", bufs=2, space="PSUM"))
        psbpool = ctx.enter_context(tc.tile_pool(name="psb", bufs=4))
        pkpool = ctx.enter_context(tc.tile_pool(name="pk", bufs=6))
        t1pool = ctx.enter_context(tc.tile_pool(name="t1", bufs=2))
        t2pool = ctx.enter_context(tc.tile_pool(name="t2", bufs=2))
        t3pool = ctx.enter_context(tc.tile_pool(name="t3", bufs=2))
        t4pool = ctx.enter_context(tc.tile_pool(name="t4", bufs=2))
        dsqpool = ctx.enter_context(tc.tile_pool(name="dsq", bufs=2))
        ixpool = ctx.enter_context(tc.tile_pool(name="ix", bufs=2))
        dmpool = ctx.enter_context(tc.tile_pool(name="dm", bufs=2))

        # DRAM views
        # z index [c, x], x = o*(8*gblk) + g*gblk + p*PW + jj
        zv = z_d[:].rearrange(
            "c (o g p jj) -> p g c o jj", o=4, g=8, p=n_pairs, jj=PW
        )
        ixv = idx_d[:].rearrange("(g b j) -> b g j", g=G3, j=2 * PW)
        dmv = dmin_d[:].rearrange("(g b j) -> b g j", g=G3, j=2 * PW)

        ix_t = dm_t = None
        for p in range(n_pairs):
            zb = zbufs[p % 2]
            nc.sync.dma_start(zb[1:121, :], zv[p])
            # z^2 -> bf16 for the zz matmuls (row 0 squares to 1.0, its
            # weight rows are zero)
            zsq = zsqpool.tile([C * 8 + 1, 4 * PW], BF16)
            nc.scalar.square(zsq[:], zb[:].bitcast(F32))

            # per-macro-half PSUM: 4 accumulating fp32r matmuls per tile
            # plus 4 bf16 z^2 matmuls for zz
            halves = []
            zzts = []
            for h in (0, 1):
                ptiles = []
                for i in range(4):
                    ps = ppool.tile([128, EFF], F32)
                    for o in range(4):
                        wsl = w_s[:, 128 * (4 * i + o) : 128 * (4 * i + o) + 128]
                        zsl = zb[:, o * PW + h * EFF : o * PW + h * EFF + EFF]
                        nc.tensor.matmul(
                            ps[:], wsl, zsl, start=(o == 0), stop=(o == 3)
                        )
                    ptiles.append(ps)
                zzt = zzpool.tile([32, EFF], F32)
                for o in range(4):
                    nc.tensor.matmul(
                        zzt[:], wz_s[:, 32 * o : 32 * o + 32],
                        zsq[:, o * PW + h * EFF : o * PW + h * EFF + EFF],
                        start=(o == 0), stop=(o == 3),
                    )
                zzts.append(zzt)
                halves.append(ptiles)

            # stage both halves into SBUF pair tiles (ACT), pack at pair
            # width in DVE 2x mode
            pks = []
            for i in range(4):
                psb = psbpool.tile([128, PW], F32)
                for h in (0, 1):
                    nc.scalar.copy(
                        psb[:, h * EFF : h * EFF + EFF], halves[h][i][:]
                    )
                pk = pkpool.tile([128, PW], F32)
                nc.vector.tensor_scalar(
                    pk[:].bitcast(I32), psb[:].bitcast(I32), AND_MASK,
                    vvec_s[:, i : i + 1],
                    op0=mybir.AluOpType.bitwise_and,
                    op1=mybir.AluOpType.bitwise_or,
                )
                pks.append(pk)

            # min tree at pair width (shifts via SBUF->SBUF DMA)
            t1a = t1pool.tile([128, PW], F32)
            nc.vector.tensor_tensor(t1a[:], pks[0][:], pks[1][:], MIN)
            t1b = t1pool.tile([128, PW], F32)
            nc.vector.tensor_tensor(t1b[:], pks[2][:], pks[3][:], MIN)
            t2 = t2pool.tile([128, PW], F32)
            nc.vector.tensor_tensor(t2[:], t1a[:], t1b[:], MIN)
            c3 = t3pool.tile([64, PW], F32, name="c3")
            nc.gpsimd.tensor_copy(c3[:], t2[64:128, :])
            t3 = t3pool.tile([64, PW], F32)
            nc.vector.tensor_tensor(t3[:], t2[0:64, :], c3[:], MIN)
            c4 = t4pool.tile([32, PW], F32, name="c4")
            nc.gpsimd.tensor_copy(c4[:], t3[32:64, :])
            t4 = t4pool.tile([32, PW], F32)
            nc.vector.tensor_tensor(t4[:], t3[0:32, :], c4[:], MIN)

            if p % 2 == 0:
                ix_t = ixpool.tile([32, 2 * PW], I32)
                dm_t = dmpool.tile([32, 2 * PW], F32)
            q = slice((p % 2) * PW, (p % 2) * PW + PW)

            # idx = packed & 15 at pair width
            nc.vector.tensor_scalar(
                ix_t[:, q], t4[:].bitcast(I32), 15, None,
                op0=mybir.AluOpType.bitwise_and,
            )
            # dmin^2 = packed + zz per half; sqrt on ACT at pair width
            dsq = dsqpool.tile([32, PW], F32)
            for h in (0, 1):
                nc.vector.tensor_tensor(
                    dsq[:, h * EFF : h * EFF + EFF],
                    t4[:, h * EFF : h * EFF + EFF], zzts[h][:], ADD,
                )
            nc.scalar.sqrt(dm_t[:, q], dsq[:])

            if p % 2 == 1:
                b = p // 2
                nc.gpsimd.dma_start(ixv[b], ix_t[:])
                nc.gpsimd.dma_start(dmv[b], dm_t[:])

    nc.compile()
    return nc


def _weights(vertices):
    import ml_dtypes

    V = np.asarray(vertices, dtype=np.float32)            # (16, 15)
    vv = (V.astype(np.float64) ** 2).sum(1).astype(np.float32)
    w = np.zeros((121, 16, 128), dtype=np.float32)
    for i in range(4):
        for o in range(4):
            blk = w[:, 4 * i + o, :]
            for g in range(8):
                g3 = 8 * o + g
                for vp in range(4):
                    blk[1 + 15 * g : 1 + 15 * g + 15, 32 * vp + g3] = (
                        -2.0 * V[4 * i + vp]
                    )
            if o == 0:
                for vp in range(4):
                    for g3 in range(32):
                        blk[0, 32 * vp + g3] = vv[4 * i + vp]
    wz = np.zeros((121, 4, 32), dtype=np.float32)
    for o in range(4):
        for g in range(8):
            wz[1 + 15 * g : 1 + 15 * g + 15, o, 8 * o + g] = 1.0
    wz = wz.astype(ml_dtypes.bfloat16)
    vvec = np.zeros((128, 4), dtype=np.int32)
    for i in range(4):
        for vp in range(4):
            vvec[32 * vp : 32 * vp + 32, i] = 4 * i + vp
    return w.reshape(121, 16 * 128), wz.reshape(121, 128), vvec


def kernel(z, vertices):
    z = np.ascontiguousarray(np.asarray(z, dtype=np.float32))
    lx, ly = z.shape[1], z.shape[2]
    n = lx * ly
    z_fl = z.reshape(C, n)
    n_loc = n // N_CORES

    if "nc" not in _CACHE:
        _CACHE["nc"] = build_nc()
    nc = _CACHE["nc"]

    w, wz, vvec = _weights(vertices)
    in_maps = []
    for c in range(N_CORES):
        in_maps.append(
            {
                "z": np.ascontiguousarray(z_fl[:, c * n_loc : (c + 1) * n_loc]),
                "w": w,
                "wz": wz,
                "vvec": vvec,
            }
        )
    res = run_bass_kernel_spmd(nc, in_maps, list(range(N_CORES)))
    X = np.concatenate([res.results[c]["idx"] for c in range(N_CORES)])
    dmin = np.concatenate([res.results[c]["dmin"] for c in range(N_CORES)])
    return X.reshape(lx, ly), dmin.reshape(lx, ly)


if __name__ == "__main__":
    print("build check")
    nc = build_nc(4)
    print("ok")


# revision 21
# speedup vs baseline: 1.8332x; 1.0314x over previous
"""Trainium2 Bass kernel for vq_codebook argmin (nn_GUMSampler).

Per pixel p, d2[v] = ||z_p - vertex_v||^2 over 16 vertices in R^15;
outputs argmin index (int32) and min distance (f32).

Layout (per core; pixels sharded 8 ways across cores, then 32 pixel
groups g3 per core with gblk = n_loc/32 pixels each; two 512-px macros
are processed together as one 1024-wide "pair"):
  - Four PSUM tiles P_i [128, 512] per macro; tile i holds vertices
    {4i+v', v'=0..3} at row 32v'+g3.  Each accumulates 4 fp32r matmuls
    (one per group-octet, K=121: ones-row 0 carries ||v||^2 on octet 0,
    then rows 1+15g+c of the shared z tile).  PSUM holds
    q = ||v||^2 - 2 v.z   (||z||^2 is argmin-irrelevant, added at the
    end from a separate small PSUM tile).
  - zz = ||z||^2 via 4 accumulating bf16 matmuls of z^2 (ACT square,
    bf16) into ZZ [32, 512] per macro.
  - ACT stages each P_i into the halves of an SBUF pair tile [128,1024];
    pack runs on DVE in 2x mode at pair width:
      packed = (bits(q) & ~15) | vertex_id   (id = 4v'+i per partition)
    f32 min then carries value AND argmin; ties pick the smaller id.
  - Min tree at pair width, every level 32-aligned:
      t1a=min(pk0,pk1) t1b=min(pk2,pk3) t2=min(t1a,t1b)  [128,1024]
      t3=min(t2[0:64], dma-shift t2[64:128])              [64,1024]
      t4=min(t3[0:32], dma-shift t3[32:64])               [32,1024]
    (walrus + HW require equal SBUF base partitions for tensor_tensor
    inputs, so the sub-64 shifts are cross-partition copies on the
    otherwise-idle GPSIMD engine.)
  - Epilogue: idx = packed & 15 (DVE 2x, pair width); dmin^2 = packed +
    zz per macro half (id junk bits <= 15 ulp, ~2e-6 rel), sqrt on ACT.
  - One z DMA per pair (one strided DMA, 480 x 4KB descriptors), one
    idx/dmin DMA per 2 pairs.
"""

import sys

sys.path.insert(0, "/opt/trn_rl_repo")

from contextlib import ExitStack

import numpy as np

import concourse.bacc as bacc
import concourse.tile as tile
from concourse import mybir
from concourse.bass_utils import run_bass_kernel_spmd

F32 = mybir.dt.float32
F32R = mybir.dt.float32r
BF16 = mybir.dt.bfloat16
I32 = mybir.dt.int32

K = 16
C = 15
G3 = 32          # pixel groups per core
EFF = 512        # pixels per macro per group
N_CORES = 8
LX = LY = 2048
N_TOTAL = LX * LY
N_LOC = N_TOTAL // N_CORES       # 524288
GBLK = N_LOC // G3               # 16384
N_MACROS = GBLK // EFF           # 32

AND_MASK = -16
MIN = mybir.AluOpType.min
ADD = mybir.AluOpType.add

_CACHE = {}


def build_nc(n_macros=N_MACROS):
    assert n_macros % 4 == 0
    gblk = n_macros * EFF
    n_loc = G3 * gblk
    n_pairs = n_macros // 2
    PW = 2 * EFF                 # pair width (1024)
    nc = bacc.Bacc("TRN2", target_bir_lowering=False, debug=False)

    z_d = nc.dram_tensor("z", [C, n_loc], F32R, kind="ExternalInput")
    w_d = nc.dram_tensor("w", [C * 8 + 1, 16 * 128], F32R, kind="ExternalInput")
    wz_d = nc.dram_tensor("wz", [C * 8 + 1, 128], BF16, kind="ExternalInput")
    vvec_d = nc.dram_tensor("vvec", [128, 4], I32, kind="ExternalInput")
    idx_d = nc.dram_tensor("idx", [n_loc], I32, kind="ExternalOutput")
    dmin_d = nc.dram_tensor("dmin", [n_loc], F32, kind="ExternalOutput")

    with tile.TileContext(nc) as tc, ExitStack() as ctx:
        cpool = ctx.enter_context(tc.tile_pool(name="consts", bufs=1))
        w_s = cpool.tile([C * 8 + 1, 16 * 128], F32R)
        wz_s = cpool.tile([C * 8 + 1, 128], BF16)
        vvec_s = cpool.tile([128, 4], I32)
        nc.sync.dma_start(w_s[:], w_d[:])
        nc.sync.dma_start(wz_s[:], wz_d[:])
        nc.sync.dma_start(vvec_s[:], vvec_d[:])

        # persistent double-buffered z pair tiles; row 0 = 1.0 (set once)
        zbufs = [
            cpool.tile([C * 8 + 1, 4 * PW], F32R, name=f"zb{k}") for k in range(2)
        ]
        for zb in zbufs:
            nc.gpsimd.memset(zb[0:1, :].bitcast(F32), 1.0)

        zsqpool = ctx.enter_context(tc.tile_pool(name="zsq", bufs=2))
        ppool = ctx.enter_context(tc.tile_pool(name="psum", bufs=6, space="PSUM"))
        zzpool = ctx.enter_context(tc.tile_pool(name="zz", bufs=2, space="PSUM"))
        psbpool = ctx.enter_context(tc.tile_pool(name="psb", bufs=4))
        pkpool = ctx.enter_context(tc.tile_pool(name="pk", bufs=6))
        t1pool = ctx.enter_context(tc.tile_pool(name="t1", bufs=2))
        t2pool = ctx.enter_context(tc.tile_pool(name="t2", bufs=2))
        t3pool = ctx.enter_context(tc.tile_pool(name="t3", bufs=2))
        t4pool = ctx.enter_context(tc.tile_pool(name="t4", bufs=2))
        dsqpool = ctx.enter_context(tc.tile_pool(name="dsq", bufs=2))
        ixpool = ctx.enter_context(tc.tile_pool(name="ix", bufs=2))
        dmpool = ctx.enter_context(tc.tile_pool(name="dm", bufs=2))

        # DRAM views
        # z index [c, x], x = o*(8*gblk) + g*gblk + p*PW + jj
        zv = z_d[:].rearrange(
            "c (o g p jj) -> p g c o jj", o=4, g=8, p=n_pairs, jj=PW
        )
        ixv = idx_d[:].rearrange("(g b j) -> b g j", g=G3, j=2 * PW)
        dmv = dmin_d[:].rearrange("(g b j) -> b g j", g=G3, j=2 * PW)

        ix_t = dm_t = None
        for p in range(n_pairs):
            zb = zbufs[p % 2]
            zsq = zsqpool.tile([C * 8 + 1, 4 * PW], BF16)
            if p == 0:
                # shorten the pipeline head: load + square octet-by-octet
                # so the first matmuls start after ~1/4 of the z transfer
                for o in range(4):
                    osl = slice(o * PW, o * PW + PW)
                    nc.sync.dma_start(zb[1:121, osl], zv[p][:, :, o])
                    nc.scalar.square(zsq[:, osl], zb[:, osl].bitcast(F32))
            else:
                nc.sync.dma_start(zb[1:121, :], zv[p])
                # z^2 -> bf16 for the zz matmuls (row 0 squares to 1.0,
                # its weight rows are zero)
                nc.scalar.square(zsq[:], zb[:].bitcast(F32))

            # per-macro-half PSUM: 4 accumulating fp32r matmuls per tile
            # plus 4 bf16 z^2 matmuls for zz
            halves = []
            zzts = []
            for h in (0, 1):
                ptiles = []
                for i in range(4):
                    ps = ppool.tile([128, EFF], F32)
                    for o in range(4):
                        wsl = w_s[:, 128 * (4 * i + o) : 128 * (4 * i + o) + 128]
                        zsl = zb[:, o * PW + h * EFF : o * PW + h * EFF + EFF]
                        nc.tensor.matmul(
                            ps[:], wsl, zsl, start=(o == 0), stop=(o == 3)
                        )
                    ptiles.append(ps)
                zzt = zzpool.tile([32, EFF], F32)
                for o in range(4):
                    nc.tensor.matmul(
                        zzt[:], wz_s[:, 32 * o : 32 * o + 32],
                        zsq[:, o * PW + h * EFF : o * PW + h * EFF + EFF],
                        start=(o == 0), stop=(o == 3),
                    )
                zzts.append(zzt)
                halves.append(ptiles)

            # stage both halves into SBUF pair tiles (ACT), pack at pair
            # width in DVE 2x mode
            pks = []
            for i in range(4):
                psb = psbpool.tile([128, PW], F32)
                for h in (0, 1):
                    nc.scalar.copy(
                        psb[:, h * EFF : h * EFF + EFF], halves[h][i][:]
                    )
                pk = pkpool.tile([128, PW], F32)
                nc.vector.tensor_scalar(
                    pk[:].bitcast(I32), psb[:].bitcast(I32), AND_MASK,
                    vvec_s[:, i : i + 1],
                    op0=mybir.AluOpType.bitwise_and,
                    op1=mybir.AluOpType.bitwise_or,
                )
                pks.append(pk)

            # min tree at pair width (shifts via SBUF->SBUF DMA)
            t1a = t1pool.tile([128, PW], F32)
            nc.vector.tensor_tensor(t1a[:], pks[0][:], pks[1][:], MIN)
            t1b = t1pool.tile([128, PW], F32)
            nc.vector.tensor_tensor(t1b[:], pks[2][:], pks[3][:], MIN)
            t2 = t2pool.tile([128, PW], F32)
            nc.vector.tensor_tensor(t2[:], t1a[:], t1b[:], MIN)
            c3 = t3pool.tile([64, PW], F32, name="c3")
            nc.gpsimd.tensor_copy(c3[:], t2[64:128, :])
            t3 = t3pool.tile([64, PW], F32)
            nc.vector.tensor_tensor(t3[:], t2[0:64, :], c3[:], MIN)
            c4 = t4pool.tile([32, PW], F32, name="c4")
            nc.gpsimd.tensor_copy(c4[:], t3[32:64, :])
            t4 = t4pool.tile([32, PW], F32)
            nc.vector.tensor_tensor(t4[:], t3[0:32, :], c4[:], MIN)

            if p % 2 == 0:
                ix_t = ixpool.tile([32, 2 * PW], I32)
                dm_t = dmpool.tile([32, 2 * PW], F32)
            q = slice((p % 2) * PW, (p % 2) * PW + PW)

            # idx = packed & 15 at pair width
            nc.vector.tensor_scalar(
                ix_t[:, q], t4[:].bitcast(I32), 15, None,
                op0=mybir.AluOpType.bitwise_and,
            )
            # dmin^2 = packed + zz per half; sqrt on ACT at pair width
            dsq = dsqpool.tile([32, PW], F32)
            for h in (0, 1):
                nc.vector.tensor_tensor(
                    dsq[:, h * EFF : h * EFF + EFF],
                    t4[:, h * EFF : h * EFF + EFF], zzts[h][:], ADD,
                )
            nc.scalar.sqrt(dm_t[:, q], dsq[:])

            if p % 2 == 1:
                b = p // 2
                nc.gpsimd.dma_start(ixv[b], ix_t[:])
                nc.gpsimd.dma_start(dmv[b], dm_t[:])

    nc.compile()
    return nc


def _weights(vertices):
    import ml_dtypes

    V = np.asarray(vertices, dtype=np.float32)            # (16, 15)
    vv = (V.astype(np.float64) ** 2).sum(1).astype(np.float32)
    w = np.zeros((121, 16, 128), dtype=np.float32)
    for i in range(4):
        for o in range(4):
            blk = w[:, 4 * i + o, :]
            for g in range(8):
                g3 = 8 * o + g
                for vp in range(4):
                    blk[1 + 15 * g : 1 + 15 * g + 15, 32 * vp + g3] = (
                        -2.0 * V[4 * i + vp]
                    )
            if o == 0:
                for vp in range(4):
                    for g3 in range(32):
                        blk[0, 32 * vp + g3] = vv[4 * i + vp]
    wz = np.zeros((121, 4, 32), dtype=np.float32)
    for o in range(4):
        for g in range(8):
            wz[1 + 15 * g : 1 + 15 * g + 15, o, 8 * o + g] = 1.0
    wz = wz.astype(ml_dtypes.bfloat16)
    vvec = np.zeros((128, 4), dtype=np.int32)
    for i in range(4):
        for vp in range(4):
            vvec[32 * vp : 32 * vp + 32, i] = 4 * i + vp
    return w.reshape(121, 16 * 128), wz.reshape(121, 128), vvec


def kernel(z, vertices):
    z = np.ascontiguousarray(np.asarray(z, dtype=np.float32))
    lx, ly = z.shape[1], z.shape[2]
    n = lx * ly
    z_fl = z.reshape(C, n)
    n_loc = n // N_CORES

    if "nc" not in _CACHE:
        _CACHE["nc"] = build_nc()
    nc = _CACHE["nc"]

    w, wz, vvec = _weights(vertices)
    in_maps = []
    for c in range(N_CORES):
        in_maps.append(
            {
                "z": np.ascontiguousarray(z_fl[:, c * n_loc : (c + 1) * n_loc]),
                "w": w,
                "wz": wz,
                "vvec": vvec,
            }
        )
    res = run_bass_kernel_spmd(nc, in_maps, list(range(N_CORES)))
    X = np.concatenate([res.results[c]["idx"] for c in range(N_CORES)])
    dmin = np.concatenate([res.results[c]["dmin"] for c in range(N_CORES)])
    return X.reshape(lx, ly), dmin.reshape(lx, ly)


if __name__ == "__main__":
    print("build check")
    nc = build_nc(4)
    print("ok")
